# revision 52
# baseline (speedup 1.0000x reference)
"""Trainium2 Bass kernel for nn_BaseRuleLearner (pair-decomposition design).

Math (per batch b, rule i; perm p=(a,b,c) distinct):
  score = F01(a,b)+F02(a,c)+F12(b,c)+g0(a)+g1(b)+g2(c), where
  Ff(x,y)  = BM[nm,x,y]+BM[mn,y,x]  for (n,m) in [(0,1),(0,2),(1,2)]
  gv(l)    = UM[v,l]+BM[vv,l,l]
  out      = softmax_i(min_p score) @ one_hot([0,0,1,1])

Stage 1 (PE): one matmul per unordered pair {j<k} with k-dim = (w,e):
  w=0 rows hold Bf[b,j,k,:], w=1 rows Bf[b,k,j,:].  Weight cols (i, d=2f+o)
  combine rb[n,m]/rb[m,n] so each output row is a full Ff value for one
  orientation: psum rows (i,d)=24 at psum partition offset 32*sl, packing
  4 pairs per [128,512] psum tile (7 tiles).  Unary+diag: 2 matmuls per
  lp-slot accumulate ru and rb-diag into one [128,512] psum tile (g-rows).

Evac: 8 copies [128,512] fp32->bf16 (vector/scalar/gpsimd) to staging.

Assembly (12 DMAs): per i, scatter staging rows into k-major Q tiles
  qt0/qt1 [96, (i,b)]: off-diag k-row r=q*6+d (chunk0 q<16), g-rows
  r=168+v*8+l (chunk1 local 72..95).

Stage 2 (PE): per (bt,i): 2 matmuls (k=96 chunks) accumulate
  psum[128b, 336p] with 0/1 gather matrix G; min over p (vector/gpsimd),
  softmax over i, pair-sum into [128,4], one gathered output DMA.
"""

import itertools
import numpy as np

B, O, E = 4096, 8, 64
I, V = 4, 3
P = 336
N_CORES = 8
BC = B // N_CORES            # 512 batch per core
NPAIR = 28                   # unordered pairs {j<k}
JBS = BC + 16                # padded block stride in staging tiles
FMAP = [(0, 1), (0, 2), (1, 2)]

_PERM = np.array(list(itertools.permutations(range(O), V)), dtype=np.int32)
_PAIRS = [(j, k) for j in range(O) for k in range(j + 1, O)]
_QIDX = {pr: q for q, pr in enumerate(_PAIRS)}

_CACHED = {}


def _build_g():
    """Gather matrix G [192, P] in k-row order r=q*6+2f+o (off-diag),
    r=168+v*8+l (g-rows); returned packed as [96, 2*P] (chunk0|chunk1)."""
    g = np.zeros((108, 2 * P), np.float32)
    for p, (a, b, c) in enumerate(_PERM):
        for f, (x, y) in enumerate(((a, b), (a, c), (b, c))):
            q, o = (_QIDX[(x, y)], 0) if x < y else (_QIDX[(y, x)], 1)
            d = f * 2 + o
            sl2, pg = q % 2, q // 2
            r = sl2 * 42 + d * 7 + (pg % 7)
            g[r, (pg // 7) * P + p] += 1.0
        for v, x in ((0, a), (1, b), (2, c)):
            g[84 + (v * 2 + x % 2) * 4 + x // 2, P + p] += 1.0
    return g  # [108, 672]: chunk0 = tiles 0-6, chunk1 = tiles 7-13 + g


def _build_w(rule_unary, rule_binary):
    """Weights [128, 88]: cols 0:24 off-diag (i*6+2f+o), 24:56 unary
    (i*8+v*2+s, 2 pad cols per i), 56:88 diag (same col order)."""
    ru = np.asarray(rule_unary, np.float32)
    rb = np.asarray(rule_binary, np.float32)
    w = np.zeros((128, 88), np.float32)
    for i in range(I):
        for f, (n, m) in enumerate(FMAP):
            for o in range(2):
                c = i * 6 + f * 2 + o
                w[0:64, c] = rb[i, n, m] if o == 0 else rb[i, m, n]
                w[64:128, c] = rb[i, m, n] if o == 0 else rb[i, n, m]
        for v in range(V):
            for s in range(2):
                c = i * 8 + v * 2 + s
                w[s * 64:(s + 1) * 64, 24 + c] = ru[i, v]
                w[s * 64:(s + 1) * 64, 56 + c] = rb[i, v, v]
    return w


def _build_module():
    import concourse.tile as tile
    from concourse import bacc, mybir

    FP = mybir.dt.float32
    BF = mybir.dt.bfloat16
    X = mybir.AxisListType.X
    nc = bacc.Bacc("TRN2", target_bir_lowering=False, debug=False)

    ao = nc.dram_tensor("ao", [128, NPAIR * BC], BF, kind="ExternalInput")
    ag = nc.dram_tensor("ag", [128, 8 * BC], BF, kind="ExternalInput")
    w = nc.dram_tensor("w", [128, 88], BF, kind="ExternalInput")
    gm = nc.dram_tensor("gm", [108, 2 * P], BF, kind="ExternalInput")
    out = nc.dram_tensor("out", [BC, 4], FP, kind="ExternalOutput")

    NBT = BC // 128

    with tile.TileContext(nc) as tc:
        with (
            tc.tile_pool(name="wpool", bufs=1) as wpool,
            tc.tile_pool(name="xpool", bufs=1) as xpool,
            tc.tile_pool(name="sgpool", bufs=1) as sgpool,
            tc.tile_pool(name="qpool", bufs=1) as qpool,
            tc.tile_pool(name="mpool", bufs=2) as mpool,
            tc.tile_pool(name="pod", bufs=3, space="PSUM") as pod,
            tc.tile_pool(name="pss", bufs=5, space="PSUM") as pss,
        ):
            # ---- input DMAs (one queue, in PE consumption order) ----
            w_sb = wpool.tile([128, 88], BF, tag="w")
            nc.sync.dma_start(w_sb[:], w.ap()[:])
            ag_sb = xpool.tile([128, 8 * BC], BF, tag="ag")
            for h in range(4):
                nc.sync.dma_start(
                    ag_sb[:, h * 2 * BC:(h + 1) * 2 * BC],
                    ag.ap()[:, h * 2 * BC:(h + 1) * 2 * BC],
                )
            ao_sb = xpool.tile([128, NPAIR * BC], BF, tag="ao")
            for ch in range(7):
                nc.sync.dma_start(
                    ao_sb[:, ch * 4 * BC:(ch + 1) * 4 * BC],
                    ao.ap()[:, ch * 4 * BC:(ch + 1) * 4 * BC],
                )
            g_sb = wpool.tile([108, 2 * P], BF, tag="g")
            nc.scalar.dma_start(g_sb[:], gm.ap()[:])

            sg_od = sgpool.tile([128, 14 * JBS], BF, tag="sgod")
            sg_g = sgpool.tile([32, 4 * JBS], BF, tag="sgg")
            qt0 = qpool.tile([84, 4 * BC], BF, tag="qt0")
            qt1 = qpool.tile([108, 4 * BC], BF, tag="qt1")

            # ---- stage 1: g (unary+diag), 2 accumulating matmuls/tile ----
            for lp in range(4):
                ps_gf = pod.tile([128, BC], FP, tag="pod")
                ps_g = ps_gf[0:32, :]
                nc.tensor.matmul(
                    ps_g, w_sb[:, 24:56],
                    ag_sb[:, 2 * lp * BC:(2 * lp + 1) * BC],
                    start=True, stop=False,
                )
                nc.tensor.matmul(
                    ps_g, w_sb[:, 56:88],
                    ag_sb[:, (2 * lp + 1) * BC:(2 * lp + 2) * BC],
                    start=False, stop=True,
                )
                if lp % 2 == 0:
                    nc.vector.tensor_copy(
                        sg_g[:, lp * JBS:lp * JBS + BC], ps_g
                    )
                else:
                    nc.scalar.copy(sg_g[:, lp * JBS:lp * JBS + BC], ps_g)

            # g-rows of qt1 can assemble as soon as sg_g is complete
            for i in range(I):
                srcg = (
                    sg_g[i * 8:i * 8 + 6, :]
                    .rearrange("p (a m) -> p a m", m=JBS)[:, :, 0:BC]
                )
                (nc.sync if i % 2 else nc.scalar).dma_start(
                    qt1[84:108, i * BC:(i + 1) * BC], srcg
                )

            # ---- stage 1: off-diag pairs, 2 per psum tile (slots 0/64) ----
            # assembly halves: tiles 0-6 scatter early, 7-13 at the end.
            # chunk0 (slot 0): k-row r = d*14 + pg -> qt0
            # chunk1 (slot 1): r = d*14 + pg, plus g rows 84 + (v,s)*4 + lp
            sgv = sg_od[:].rearrange("(s r) (a m) -> s r a m", s=2, m=JBS)

            def emit_asm(half):
                # half 0 -> qt0 (tiles 0-6), half 1 -> qt1 (tiles 7-13).
                # half 0 avoids sync: its queue still streams inputs then.
                qeng = [nc.gpsimd, nc.scalar] if half == 0 else [
                    nc.sync, nc.gpsimd, nc.scalar
                ]
                nq = len(qeng)
                lo, hi = (0, 7) if half == 0 else (7, 14)
                qt = qt0 if half == 0 else qt1
                for i in range(I):
                    qeng[(2 * i) % nq].dma_start(
                        qt[0:42, i * BC:(i + 1) * BC],
                        sgv[0, i * 6:i * 6 + 6, lo:hi, 0:BC],
                    )
                    qeng[(2 * i + 1) % nq].dma_start(
                        qt[42:84, i * BC:(i + 1) * BC],
                        sgv[1, i * 6:i * 6 + 6, lo:hi, 0:BC],
                    )

            # c0 matmuls for i=0 pre-run in the input-starved tail gaps
            pre_sc = {}
            for pg in range(14):
                ps = pod.tile([128, BC], FP, tag="pod")
                for sl in range(2):
                    q = pg * 2 + sl
                    nc.tensor.matmul(
                        ps[64 * sl:64 * sl + 24, :],
                        w_sb[:, 0:24],
                        ao_sb[:, q * BC:(q + 1) * BC],
                        start=True, stop=True, tile_position=(0, 64 * sl),
                    )
                dst = sg_od[:, pg * JBS:pg * JBS + BC]
                if pg % 2 == 0:
                    nc.vector.tensor_copy(dst, ps[:])
                else:
                    nc.scalar.copy(dst, ps[:])
                if pg == 6:
                    emit_asm(0)
                if 9 <= pg <= 12:
                    bt = pg - 9
                    sc = pss.tile([128, P], FP, tag="sc")
                    nc.tensor.matmul(
                        sc[:], qt0[:, bt * 128:bt * 128 + 128],
                        g_sb[0:84, 0:P], start=True, stop=False,
                    )
                    pre_sc[(0, bt)] = sc
            emit_asm(1)

            # ---- stage 2: scores + min (i-outer), then softmax per bt ----
            fin = mpool.tile([128, 4 * NBT], FP, tag="fin", bufs=1)
            merged = mpool.tile([128, 4 * NBT], FP, tag="m", bufs=1)
            nc.vector.memset(fin[:], 0.0)

            def softmax(bt):
                # softmax over i (exp without max-shift: scores are O(10),
                # safe in fp32), pair-sum (one_hot([0,0,1,1])) and scale
                mview = merged[:, bt * 4:bt * 4 + 4]
                ex = mpool.tile([128, 4], FP, tag="ex")
                sm = mpool.tile([128, 1], FP, tag="sm")
                nc.scalar.activation(
                    ex[:], mview, mybir.ActivationFunctionType.Exp,
                    accum_out=sm[:],
                )
                rc = mpool.tile([128, 1], FP, tag="rc")
                nc.vector.reciprocal(rc[:], sm[:])
                ea = mpool.tile([128, 2], FP, tag="ea")
                ex3 = ex[:].rearrange("p (a b) -> p a b", b=2)
                nc.vector.tensor_add(ea[:], ex3[:, :, 0], ex3[:, :, 1])
                nc.vector.tensor_scalar_mul(
                    fin[:, bt * 4:bt * 4 + 2], ea[:], rc[:]
                )

            order = [(0, bt) for bt in range(NBT)] + [
                (i, bt) for bt in range(NBT) for i in range(1, I)
            ]
            for i, bt in order:
                sc = pre_sc.get((i, bt))
                col = i * BC + bt * 128
                if sc is None:
                    sc = pss.tile([128, P], FP, tag="sc")
                    nc.tensor.matmul(
                        sc[:], qt0[:, col:col + 128], g_sb[0:84, 0:P],
                        start=True, stop=False,
                    )
                nc.tensor.matmul(
                    sc[:], qt1[:, col:col + 128], g_sb[:, P:2 * P],
                    start=False, stop=True,
                )
                nc.vector.tensor_reduce(
                    merged[:, bt * 4 + i:bt * 4 + i + 1], sc[:], axis=X,
                    op=mybir.AluOpType.min,
                )
                if i == I - 1:
                    softmax(bt)
            outv = out.ap().rearrange("(a p) m -> p a m", p=128)
            nc.sync.dma_start(outv, fin[:].rearrange("p (a m) -> p a m", a=NBT))

    nc.compile()
    return nc


def _get_module():
    if "nc" not in _CACHED:
        _CACHED["nc"] = _build_module()
    return _CACHED["nc"]


def _host_inputs(unary_feats, binary_feats, rule_unary, rule_binary):
    import ml_dtypes

    bf16 = ml_dtypes.bfloat16
    uf = np.asarray(unary_feats, np.float32)
    bf = np.asarray(binary_feats, np.float32)

    w = _build_w(rule_unary, rule_binary).astype(bf16)
    g = _build_g().astype(bf16)
    jj = np.array([p[0] for p in _PAIRS])
    kk = np.array([p[1] for p in _PAIRS])

    in_maps = []
    for c in range(N_CORES):
        bfc = bf[c * BC:(c + 1) * BC]                    # [BC, O, O, E]
        ufc = uf[c * BC:(c + 1) * BC]                    # [BC, O, E]
        ao = np.empty((128, NPAIR * BC), np.float32)
        ao[0:64] = bfc[:, jj, kk, :].transpose(2, 1, 0).reshape(64, -1)
        ao[64:128] = bfc[:, kk, jj, :].transpose(2, 1, 0).reshape(64, -1)
        # ag: interleaved (au_lp | ad_lp) blocks; rows (s, e)
        au = ufc.reshape(BC, 4, 2, E).transpose(2, 3, 1, 0)     # [2,E,4,BC]
        dg = bfc[:, np.arange(O), np.arange(O), :]              # [BC, O, E]
        ad = dg.reshape(BC, 4, 2, E).transpose(2, 3, 1, 0)      # [2,E,4,BC]
        ag = np.stack([au, ad], axis=3).reshape(128, -1)        # [(s,e),(lp,u|d,b)]
        in_maps.append({
            "ao": ao.astype(bf16), "ag": ag.astype(bf16), "w": w, "gm": g,
        })
    return in_maps


TRACE = False  # set True (e.g. from test.py) to capture an NTFF profile


def kernel(unary_feats, binary_feats, rule_unary, rule_binary):
    from concourse.bass_utils import run_bass_kernel_spmd

    nc = _get_module()
    in_maps = _host_inputs(unary_feats, binary_feats, rule_unary, rule_binary)
    res = run_bass_kernel_spmd(
        nc, in_maps, core_ids=list(range(N_CORES)), trace=TRACE
    )
    _CACHED["last_results"] = res
    return np.concatenate(
        [res.results[c]["out"] for c in range(N_CORES)], axis=0
    )


# revision 53
# speedup vs baseline: 1.0797x; 1.0797x over previous
"""Trainium2 Bass kernel for nn_BaseRuleLearner (pair-decomposition design).

Math (per batch b, rule i; perm p=(a,b,c) distinct):
  score = F01(a,b)+F02(a,c)+F12(b,c)+g0(a)+g1(b)+g2(c), where
  Ff(x,y)  = BM[nm,x,y]+BM[mn,y,x]  for (n,m) in [(0,1),(0,2),(1,2)]
  gv(l)    = UM[v,l]+BM[vv,l,l]
  out      = softmax_i(min_p score) @ one_hot([0,0,1,1])

Stage 1 (PE): one matmul per unordered pair {j<k} with k-dim = (w,e):
  w=0 rows hold Bf[b,j,k,:], w=1 rows Bf[b,k,j,:].  Weight cols (i, d=2f+o)
  combine rb[n,m]/rb[m,n] so each output row is a full Ff value for one
  orientation: psum rows (i,d)=24 at psum partition offset 32*sl, packing
  4 pairs per [128,512] psum tile (7 tiles).  Unary+diag: 2 matmuls per
  lp-slot accumulate ru and rb-diag into one [128,512] psum tile (g-rows).

Evac: 8 copies [128,512] fp32->bf16 (vector/scalar/gpsimd) to staging.

Assembly (12 DMAs): per i, scatter staging rows into k-major Q tiles
  qt0/qt1 [96, (i,b)]: off-diag k-row r=q*6+d (chunk0 q<16), g-rows
  r=168+v*8+l (chunk1 local 72..95).

Stage 2 (PE): per (bt,i): 2 matmuls (k=96 chunks) accumulate
  psum[128b, 336p] with 0/1 gather matrix G; min over p (vector/gpsimd),
  softmax over i, pair-sum into [128,4], one gathered output DMA.
"""

import itertools
import numpy as np

B, O, E = 4096, 8, 64
I, V = 4, 3
P = 336
N_CORES = 8
BC = B // N_CORES            # 512 batch per core
NPAIR = 28                   # unordered pairs {j<k}
JBS = BC + 16                # padded block stride in staging tiles
FMAP = [(0, 1), (0, 2), (1, 2)]

_PERM = np.array(list(itertools.permutations(range(O), V)), dtype=np.int32)
_PAIRS = [(j, k) for j in range(O) for k in range(j + 1, O)]
_QIDX = {pr: q for q, pr in enumerate(_PAIRS)}

_CACHED = {}


def _build_g():
    """Gather matrix G [192, P] in k-row order r=q*6+2f+o (off-diag),
    r=168+v*8+l (g-rows); returned packed as [96, 2*P] (chunk0|chunk1)."""
    g = np.zeros((108, 2 * P), np.float32)
    for p, (a, b, c) in enumerate(_PERM):
        for f, (x, y) in enumerate(((a, b), (a, c), (b, c))):
            q, o = (_QIDX[(x, y)], 0) if x < y else (_QIDX[(y, x)], 1)
            d = f * 2 + o
            sl2, pg = q % 2, q // 2
            r = sl2 * 42 + d * 7 + (pg % 7)
            g[r, (pg // 7) * P + p] += 1.0
        for v, x in ((0, a), (1, b), (2, c)):
            g[84 + (v * 2 + x % 2) * 4 + x // 2, P + p] += 1.0
    return g  # [108, 672]: chunk0 = tiles 0-6, chunk1 = tiles 7-13 + g


def _build_w(rule_unary, rule_binary):
    """Weights [128, 88]: cols 0:24 off-diag (i*6+2f+o), 24:56 unary
    (i*8+v*2+s, 2 pad cols per i), 56:88 diag (same col order)."""
    ru = np.asarray(rule_unary, np.float32)
    rb = np.asarray(rule_binary, np.float32)
    w = np.zeros((128, 88), np.float32)
    for i in range(I):
        for f, (n, m) in enumerate(FMAP):
            for o in range(2):
                c = i * 6 + f * 2 + o
                w[0:64, c] = rb[i, n, m] if o == 0 else rb[i, m, n]
                w[64:128, c] = rb[i, m, n] if o == 0 else rb[i, n, m]
        for v in range(V):
            for s in range(2):
                c = i * 8 + v * 2 + s
                w[s * 64:(s + 1) * 64, 24 + c] = ru[i, v]
                w[s * 64:(s + 1) * 64, 56 + c] = rb[i, v, v]
    return w


def _build_module():
    import concourse.tile as tile
    from concourse import bacc, mybir

    FP = mybir.dt.float32
    BF = mybir.dt.bfloat16
    X = mybir.AxisListType.X
    nc = bacc.Bacc("TRN2", target_bir_lowering=False, debug=False)

    ao = nc.dram_tensor("ao", [128, NPAIR * BC], BF, kind="ExternalInput")
    ag = nc.dram_tensor("ag", [128, 8 * BC], BF, kind="ExternalInput")
    w = nc.dram_tensor("w", [128, 88], BF, kind="ExternalInput")
    gm = nc.dram_tensor("gm", [108, 2 * P], BF, kind="ExternalInput")
    out = nc.dram_tensor("out", [BC, 4], FP, kind="ExternalOutput")

    NBT = BC // 128

    with tile.TileContext(nc) as tc:
        with (
            tc.tile_pool(name="wpool", bufs=1) as wpool,
            tc.tile_pool(name="xpool", bufs=1) as xpool,
            tc.tile_pool(name="sgpool", bufs=1) as sgpool,
            tc.tile_pool(name="qpool", bufs=1) as qpool,
            tc.tile_pool(name="mpool", bufs=2) as mpool,
            tc.tile_pool(name="pod", bufs=3, space="PSUM") as pod,
            tc.tile_pool(name="pss", bufs=5, space="PSUM") as pss,
        ):
            # ---- input DMAs (one queue, in PE consumption order) ----
            w_sb = wpool.tile([128, 88], BF, tag="w")
            nc.sync.dma_start(w_sb[:], w.ap()[:])
            ag_sb = xpool.tile([128, 8 * BC], BF, tag="ag")
            for h in range(4):
                nc.sync.dma_start(
                    ag_sb[:, h * 2 * BC:(h + 1) * 2 * BC],
                    ag.ap()[:, h * 2 * BC:(h + 1) * 2 * BC],
                )
            ao_sb = xpool.tile([128, NPAIR * BC], BF, tag="ao")
            for ch in range(7):
                nc.sync.dma_start(
                    ao_sb[:, ch * 4 * BC:(ch + 1) * 4 * BC],
                    ao.ap()[:, ch * 4 * BC:(ch + 1) * 4 * BC],
                )
            g_sb = wpool.tile([108, 2 * P], BF, tag="g")
            nc.scalar.dma_start(g_sb[:], gm.ap()[:])

            sg_od = sgpool.tile([128, 14 * JBS], BF, tag="sgod")
            sg_g = sgpool.tile([32, 4 * JBS], BF, tag="sgg")
            qt0 = qpool.tile([84, 4 * BC], BF, tag="qt0")
            qt1 = qpool.tile([108, 4 * BC], BF, tag="qt1")

            # ---- stage 1: g (unary+diag), 2 accumulating matmuls/tile ----
            for lp in range(4):
                ps_gf = pod.tile([128, BC], FP, tag="pod")
                ps_g = ps_gf[0:32, :]
                nc.tensor.matmul(
                    ps_g, w_sb[:, 24:56],
                    ag_sb[:, 2 * lp * BC:(2 * lp + 1) * BC],
                    start=True, stop=False,
                )
                nc.tensor.matmul(
                    ps_g, w_sb[:, 56:88],
                    ag_sb[:, (2 * lp + 1) * BC:(2 * lp + 2) * BC],
                    start=False, stop=True,
                )
                if lp % 2 == 0:
                    nc.vector.tensor_copy(
                        sg_g[:, lp * JBS:lp * JBS + BC], ps_g
                    )
                else:
                    nc.scalar.copy(sg_g[:, lp * JBS:lp * JBS + BC], ps_g)

            # g-rows of qt1 can assemble as soon as sg_g is complete
            for i in range(I):
                srcg = (
                    sg_g[i * 8:i * 8 + 6, :]
                    .rearrange("p (a m) -> p a m", m=JBS)[:, :, 0:BC]
                )
                (nc.sync if i % 2 else nc.scalar).dma_start(
                    qt1[84:108, i * BC:(i + 1) * BC], srcg
                )

            # ---- stage 1: off-diag pairs, 2 per psum tile (slots 0/64) ----
            # assembly halves: tiles 0-6 scatter early, 7-13 at the end.
            # chunk0 (slot 0): k-row r = d*14 + pg -> qt0
            # chunk1 (slot 1): r = d*14 + pg, plus g rows 84 + (v,s)*4 + lp
            sgv = sg_od[:].rearrange("(s r) (a m) -> s r a m", s=2, m=JBS)

            qeng = [nc.sync, nc.gpsimd, nc.scalar]
            nq = 3

            def emit_asm(half):
                # half 0 -> qt0 (tiles 0-6), half 1 -> qt1 (tiles 7-13)
                lo, hi = (0, 7) if half == 0 else (7, 14)
                qt = qt0 if half == 0 else qt1
                for i in range(I):
                    qeng[(2 * i) % nq].dma_start(
                        qt[0:42, i * BC:(i + 1) * BC],
                        sgv[0, i * 6:i * 6 + 6, lo:hi, 0:BC],
                    )
                    qeng[(2 * i + 1) % nq].dma_start(
                        qt[42:84, i * BC:(i + 1) * BC],
                        sgv[1, i * 6:i * 6 + 6, lo:hi, 0:BC],
                    )

            # c0 matmuls for i=0 pre-run in the input-starved tail gaps
            pre_sc = {}
            for pg in range(14):
                ps = pod.tile([128, BC], FP, tag="pod")
                for sl in range(2):
                    q = pg * 2 + sl
                    nc.tensor.matmul(
                        ps[64 * sl:64 * sl + 24, :],
                        w_sb[:, 0:24],
                        ao_sb[:, q * BC:(q + 1) * BC],
                        start=True, stop=True, tile_position=(0, 64 * sl),
                    )
                dst = sg_od[:, pg * JBS:pg * JBS + BC]
                if pg % 2 == 0:
                    nc.vector.tensor_copy(dst, ps[:])
                else:
                    nc.scalar.copy(dst, ps[:])
                if pg == 6:
                    emit_asm(0)
                if 9 <= pg <= 12:
                    bt = pg - 9
                    sc = pss.tile([128, P], FP, tag="sc")
                    nc.tensor.matmul(
                        sc[:], qt0[:, bt * 128:bt * 128 + 128],
                        g_sb[0:84, 0:P], start=True, stop=False,
                    )
                    pre_sc[(0, bt)] = sc
            emit_asm(1)

            # ---- stage 2: scores + min (i-outer), then softmax per bt ----
            fin = mpool.tile([128, 4 * NBT], FP, tag="fin", bufs=1)
            merged = mpool.tile([128, 4 * NBT], FP, tag="m", bufs=1)
            nc.vector.memset(fin[:], 0.0)

            def softmax(bt):
                # softmax over i (exp without max-shift: scores are O(10),
                # safe in fp32), pair-sum (one_hot([0,0,1,1])) and scale
                mview = merged[:, bt * 4:bt * 4 + 4]
                ex = mpool.tile([128, 4], FP, tag="ex")
                sm = mpool.tile([128, 1], FP, tag="sm")
                nc.scalar.activation(
                    ex[:], mview, mybir.ActivationFunctionType.Exp,
                    accum_out=sm[:],
                )
                rc = mpool.tile([128, 1], FP, tag="rc")
                nc.vector.reciprocal(rc[:], sm[:])
                ea = mpool.tile([128, 2], FP, tag="ea")
                ex3 = ex[:].rearrange("p (a b) -> p a b", b=2)
                nc.vector.tensor_add(ea[:], ex3[:, :, 0], ex3[:, :, 1])
                nc.vector.tensor_scalar_mul(
                    fin[:, bt * 4:bt * 4 + 2], ea[:], rc[:]
                )

            order = [(0, bt) for bt in range(NBT)] + [
                (i, bt) for bt in range(NBT) for i in range(1, I)
            ]
            for i, bt in order:
                sc = pre_sc.get((i, bt))
                col = i * BC + bt * 128
                if sc is None:
                    sc = pss.tile([128, P], FP, tag="sc")
                    nc.tensor.matmul(
                        sc[:], qt0[:, col:col + 128], g_sb[0:84, 0:P],
                        start=True, stop=False,
                    )
                nc.tensor.matmul(
                    sc[:], qt1[:, col:col + 128], g_sb[:, P:2 * P],
                    start=False, stop=True,
                )
                nc.vector.tensor_reduce(
                    merged[:, bt * 4 + i:bt * 4 + i + 1], sc[:], axis=X,
                    op=mybir.AluOpType.min,
                )
                if i == I - 1:
                    softmax(bt)
            outv = out.ap().rearrange("(a p) m -> p a m", p=128)
            nc.sync.dma_start(outv, fin[:].rearrange("p (a m) -> p a m", a=NBT))

    nc.compile()
    return nc


def _get_module():
    if "nc" not in _CACHED:
        _CACHED["nc"] = _build_module()
    return _CACHED["nc"]


def _host_inputs(unary_feats, binary_feats, rule_unary, rule_binary):
    import ml_dtypes

    bf16 = ml_dtypes.bfloat16
    uf = np.asarray(unary_feats, np.float32)
    bf = np.asarray(binary_feats, np.float32)

    w = _build_w(rule_unary, rule_binary).astype(bf16)
    g = _build_g().astype(bf16)
    jj = np.array([p[0] for p in _PAIRS])
    kk = np.array([p[1] for p in _PAIRS])

    in_maps = []
    for c in range(N_CORES):
        bfc = bf[c * BC:(c + 1) * BC]                    # [BC, O, O, E]
        ufc = uf[c * BC:(c + 1) * BC]                    # [BC, O, E]
        ao = np.empty((128, NPAIR * BC), np.float32)
        ao[0:64] = bfc[:, jj, kk, :].transpose(2, 1, 0).reshape(64, -1)
        ao[64:128] = bfc[:, kk, jj, :].transpose(2, 1, 0).reshape(64, -1)
        # ag: interleaved (au_lp | ad_lp) blocks; rows (s, e)
        au = ufc.reshape(BC, 4, 2, E).transpose(2, 3, 1, 0)     # [2,E,4,BC]
        dg = bfc[:, np.arange(O), np.arange(O), :]              # [BC, O, E]
        ad = dg.reshape(BC, 4, 2, E).transpose(2, 3, 1, 0)      # [2,E,4,BC]
        ag = np.stack([au, ad], axis=3).reshape(128, -1)        # [(s,e),(lp,u|d,b)]
        in_maps.append({
            "ao": ao.astype(bf16), "ag": ag.astype(bf16), "w": w, "gm": g,
        })
    return in_maps


TRACE = False  # set True (e.g. from test.py) to capture an NTFF profile


def kernel(unary_feats, binary_feats, rule_unary, rule_binary):
    from concourse.bass_utils import run_bass_kernel_spmd

    nc = _get_module()
    in_maps = _host_inputs(unary_feats, binary_feats, rule_unary, rule_binary)
    res = run_bass_kernel_spmd(
        nc, in_maps, core_ids=list(range(N_CORES)), trace=TRACE
    )
    _CACHED["last_results"] = res
    return np.concatenate(
        [res.results[c]["out"] for c in range(N_CORES)], axis=0
    )
